# revision 29
# baseline (speedup 1.0000x reference)
"""GQA attention kernel for Trainium2, 8 NeuronCores — v3.

Problem: B=1, S=4096, HIDDEN=2048, 8 query heads x d=256, 1 shared KV head,
causal mask, fp32 in/out.

Sharding: full tensor-parallel over heads with a replicated input.
Host-side, x is transposed/replicated to all cores in bf16 with an SBUF-
matched layout so every big load is ONE coalesced DMA (DMA dispatch on the
sync queue costs ~0.6us per instruction, so few/fat DMAs matter).
Core j owns head j end-to-end:
  1. stream xT in 4 chunks of 1024 rows; project q (own head only) and kv
     (replicated — cheaper than the AllGather it replaces) with N=1024
     bf16 matmuls; PE-transpose kv into rows-major for the AV stationary.
  2. causal flash attention for head j over all 4096 rows (fp32 PSUM, no
     max subtraction), diagonal blocks narrowed to the causal range.
  3. normalize, one bf16 AllToAll (head-major -> row-block-major).
  4. output projection of own 512-row block against full wo (N=1024).
Host concatenates the 8 row blocks.
"""

import sys

import numpy as np

sys.path.insert(0, "/opt/trn_rl_repo")

S = 4096
HID = 2048
NH = 8
D = 256
NCORES = 8
R = 512  # output rows per core / q rows per attention block
CH = 1024  # projection chunk rows
NCH = S // CH
NEG = -1.0e9
SCALE = 1.0 / 16.0  # 1/sqrt(256)

_BUILT = None


def _build():
    global _BUILT
    if _BUILT is not None:
        return _BUILT

    from contextlib import ExitStack

    import ml_dtypes

    from concourse import bacc, tile
    from concourse.bass import mybir

    dt = mybir.dt
    f32 = dt.float32
    bf16 = dt.bfloat16
    bfnp = ml_dtypes.bfloat16
    AF = mybir.ActivationFunctionType

    nc = bacc.Bacc(
        "TRN2",
        target_bir_lowering=False,
        debug=False,
        num_devices=NCORES,
    )

    # ---- DRAM I/O (host-side layouts matched to SBUF tiles) ----
    # xT_d[c, p, hs*CH+col] = x[CH*c+col, 128*hs+p]
    xT_d = nc.dram_tensor("xT", [NCH, 128, 16 * CH], bf16, kind="ExternalInput")
    # wqkv_d[p, hs*512 + j] = (wq_head | wkv)[128*hs+p, j]  (j<256 -> wq)
    wqkv_d = nc.dram_tensor("wqkv", [128, 16 * 512], bf16, kind="ExternalInput")
    # bqkv_d columns: [bq_dh0, bq_dh1, bkv_dh0, bkv_dh1]
    bqkv_d = nc.dram_tensor("bqkv", [128, 4], f32, kind="ExternalInput")
    # wo_d[p, k*HID + col] = wo2[(128*k+p), col]
    wo_d = nc.dram_tensor("wo2d", [128, 16 * HID], bf16, kind="ExternalInput")
    bo_row = nc.dram_tensor("bo_row", [1, HID], bf16, kind="ExternalInput")
    out = nc.dram_tensor("out", [R, HID], f32, kind="ExternalOutput")

    # ---- collective buffers ----
    grp = [list(range(NCORES))]
    ao_send = nc.dram_tensor("ao_send", [NH * D, R], bf16)
    ao_recv = nc.dram_tensor("ao_recv", [NH * D, R], bf16)

    # ---- compile-time constants ----
    fp8 = dt.float8e4
    fp8np = ml_dtypes.float8_e4m3fn
    ident_np = np.eye(128).astype(bfnp)
    ones_col_np = np.ones((128, 1)).astype(bfnp)
    ones_row_np = np.ones((1, 128)).astype(bfnp)
    ones8_np = np.ones((128, 2, 32), dtype=fp8np)
    # one shared diagonal mask: every diagonal (grel, sl) sub-block equals
    # m0[kappa, col - rel0] with m0[kappa, c] = NEG iff kappa > c
    kappa = np.arange(128)[:, None]
    cols = np.arange(512)[None, :]
    mask_np = np.where(kappa <= cols, 0.0, NEG).astype(np.float32)
    ident_d = nc.inline_tensor(ident_np, "ident")
    ones_col_d = nc.inline_tensor(ones_col_np, "ones_col")
    ones_row_d = nc.inline_tensor(ones_row_np, "ones_row")
    ones8_d = nc.inline_tensor(ones8_np, "ones8")
    mask_d = nc.inline_tensor(mask_np, "mask_const")

    with tile.TileContext(nc) as tc:
        with ExitStack() as top:
            wo_pool = top.enter_context(tc.tile_pool(name="wo", bufs=1))
            cpool = top.enter_context(tc.tile_pool(name="const", bufs=1))

            # pools alive through projections + attention (closed before
            # phase 3 to make room for aoT/osb)
            big_stack = top.enter_context(ExitStack())
            big = big_stack.enter_context(tc.tile_pool(name="big", bufs=1))
            qT = big.tile([128, 2 * S], bf16, tag="qT")  # [d-slice, rows]
            kvT = big.tile([128, 2 * S], bf16, tag="kvT")  # [d-slice, keys]
            kv_sb = big.tile([128, 32 * D], bf16, tag="kv")  # rows-major kv
            mask_sb = big.tile([128, 512], f32, tag="mask")

            with ExitStack() as ph1:
                wpool = ph1.enter_context(tc.tile_pool(name="w", bufs=1))
                xr_pool = ph1.enter_context(tc.tile_pool(name="xr", bufs=2))

                # critical-path loads first: weights, then x chunks 0/1
                wqkv_sb = wpool.tile([128, 16 * 512], bf16, tag="wqkv")
                nc.sync.dma_start(wqkv_sb[:], wqkv_d[:])
                xrs = []
                for c in range(2):
                    xr = xr_pool.tile([128, 16 * CH], bf16, tag="xr", name=f"xr{c}")
                    nc.scalar.dma_start(xr[:], xT_d[c])
                    xrs.append(xr)

                # remaining constants
                ident = cpool.tile([128, 128], bf16, tag="ident")
                nc.sync.dma_start(ident[:], ident_d[:])
                ones_col = cpool.tile([128, 1], bf16, tag="ones_col")
                nc.sync.dma_start(ones_col[:], ones_col_d[:])
                ones_row = cpool.tile([1, 128], bf16, tag="ones_row")
                nc.sync.dma_start(ones_row[:], ones_row_d[:])
                ones8 = cpool.tile([128, 2, 32], fp8, tag="ones8")
                nc.sync.dma_start(ones8[:], ones8_d[:])
                bqkv_sb = cpool.tile([128, 4], f32, tag="bqkv")
                nc.sync.dma_start(bqkv_sb[:], bqkv_d[:])
                nc.sync.dma_start(mask_sb[:], mask_d[:])
                bor_sb = cpool.tile([1, HID], bf16, tag="bor")
                nc.sync.dma_start(bor_sb[:], bo_row[:])

                # ============ phase 1: projections (4 chunks of 1024) ========
                pj_psum = ph1.enter_context(
                    tc.tile_pool(name="pj_psum", bufs=6, space="PSUM")
                )
                tp_psum = ph1.enter_context(
                    tc.tile_pool(name="tp_psum", bufs=2, space="PSUM")
                )
                for c in range(NCH):
                    if c < 2:
                        xr = xrs[c]
                    else:
                        xr = xr_pool.tile(
                            [128, 16 * CH], bf16, tag="xr", name=f"xr{c}"
                        )
                        nc.scalar.dma_start(xr[:], xT_d[c])
                    for dh in range(2):
                        for wofs, dst, bcol in (
                            (0, qT, 0),
                            (256, kvT, 2),
                        ):
                            for half in range(CH // 512):
                                pp = pj_psum.tile([128, 512], f32, tag="pj")
                                for hs in range(16):
                                    nc.tensor.matmul(
                                        pp[:],
                                        wqkv_sb[
                                            :,
                                            512 * hs + wofs + 128 * dh : 512 * hs
                                            + wofs
                                            + 128 * dh
                                            + 128,
                                        ],
                                        xr[
                                            :,
                                            CH * hs + 512 * half : CH * hs
                                            + 512 * half
                                            + 512,
                                        ],
                                        start=(hs == 0),
                                        stop=(hs == 15),
                                    )
                                nc.vector.tensor_scalar_add(
                                    dst[
                                        :,
                                        S * dh + CH * c + 512 * half : S * dh
                                        + CH * c
                                        + 512 * half
                                        + 512,
                                    ],
                                    pp[:],
                                    bqkv_sb[:, bcol + dh : bcol + dh + 1],
                                )
                    # rows-major kv via PE transpose (bf16)
                    for i4 in range(CH // 128):
                        kt = (CH // 128) * c + i4
                        for dh in range(2):
                            tp = tp_psum.tile([128, 128], bf16, tag="tp")
                            nc.tensor.transpose(
                                tp[:],
                                kvT[
                                    :,
                                    S * dh + CH * c + 128 * i4 : S * dh
                                    + CH * c
                                    + 128 * i4
                                    + 128,
                                ],
                                ident[:],
                            )
                            nc.scalar.copy(
                                kv_sb[:, D * kt + 128 * dh : D * kt + 128 * dh + 128],
                                tp[:],
                            )

            # prefetch wo during attention (one coalesced DMA)
            wo_sb = wo_pool.tile([128, 16 * HID], bf16, tag="wo")
            nc.scalar.dma_start(wo_sb[:], wo_d[:])

            # ============ phase 2: causal flash attention ============
            with ExitStack() as ph2:
                s_psum = ph2.enter_context(
                    tc.tile_pool(name="s_psum", bufs=3, space="PSUM")
                )
                ao_psum = ph2.enter_context(
                    tc.tile_pool(name="ao_psum", bufs=3, space="PSUM")
                )
                den_psum = ph2.enter_context(
                    tc.tile_pool(name="den_psum", bufs=2, space="PSUM")
                )
                p_pool = ph2.enter_context(tc.tile_pool(name="p", bufs=3))
                p8_pool = ph2.enter_context(tc.tile_pool(name="p8", bufs=3))
                nrm_pool = ph2.enter_context(tc.tile_pool(name="nrm", bufs=2))
                aon_pool = ph2.enter_context(tc.tile_pool(name="aon", bufs=4))

                for b in range(8):
                    aops = [
                        ao_psum.tile([128, R], f32, tag="aops", name=f"aops{b}_{i}")
                        for i in range(2)
                    ]
                    denp = den_psum.tile([32, R], f32, tag="denp")
                    ngroups = 2 * (b + 1)

                    def consume(item):
                        kg, sl, r0, pt_c, pt8_c = item
                        k = 2 * kg + sl
                        diag_c = kg >= 2 * b
                        first = kg == 0 and sl == 0
                        last = kg == ngroups - 1 and sl == 1
                        if diag_c:
                            nc.tensor.matmul(
                                denp[0:1, r0:R],
                                ones_col[:],
                                pt_c[:, 512 * sl + r0 : 512 * sl + 512],
                                start=first,
                                stop=last,
                                skip_group_check=True,
                            )
                        for dh in range(2):
                            nc.tensor.matmul(
                                aops[dh][:, r0:R],
                                kv_sb[:, D * k + 128 * dh : D * k + 128 * dh + 128],
                                pt_c[:, 512 * sl + r0 : 512 * sl + 512],
                                start=first,
                                stop=last,
                            )
                        if (not diag_c) and sl == 1:
                            # denominator: one fp8 DoubleRow matmul per group
                            # (den is a coherent positive sum — fp8 rounding
                            # averages out)
                            nc.tensor.matmul(
                                denp[:, 0:R],
                                ones8[:],
                                pt8_c[:],
                                start=(kg == 0),
                                stop=False,
                                perf_mode=mybir.MatmulPerfMode.DoubleRow,
                                skip_group_check=True,
                            )

                    prev = None
                    pt = None
                    pt8 = None
                    for kg in range(ngroups):
                        diag = kg >= 2 * b
                        grel = kg - 2 * b
                        for sl in range(2):
                            r0 = 256 * grel + 128 * sl if diag else 0
                            k = 2 * kg + sl
                            st = s_psum.tile([128, 512], f32, tag="st")
                            for dh in range(2):
                                nc.tensor.matmul(
                                    st[:, r0:512],
                                    kvT[:, S * dh + 128 * k : S * dh + 128 * k + 128],
                                    qT[:, S * dh + R * b + r0 : S * dh + R * b + R],
                                    start=(dh == 0),
                                    stop=(dh == 1),
                                )
                            if diag:
                                nc.vector.tensor_add(
                                    st[:, r0:512],
                                    st[:, r0:512],
                                    mask_sb[:, 0 : 512 - r0],
                                )
                            if sl == 0:
                                pt = p_pool.tile([128, 1024], bf16, tag="pt")
                                pt8 = (
                                    None
                                    if diag
                                    else p8_pool.tile([128, 2, 512], fp8, tag="pt8")
                                )
                            nc.scalar.activation(
                                pt[:, 512 * sl + r0 : 512 * sl + 512],
                                st[:, r0:512],
                                AF.Exp,
                                scale=SCALE,
                            )
                            if not diag:
                                nc.scalar.activation(
                                    pt8[:, sl, :],
                                    st[:, 0:512],
                                    AF.Exp,
                                    scale=SCALE,
                                )
                            if prev is not None:
                                consume(prev)
                            prev = (kg, sl, r0, pt, pt8)
                    consume(prev)
                    # normalize + send (bf16)
                    den_sb = nrm_pool.tile([1, R], f32, tag="den_sb")
                    nc.vector.reciprocal(den_sb[:], denp[0:1, :])
                    bc = nrm_pool.tile([128, R], f32, tag="bc")
                    nc.gpsimd.partition_broadcast(bc[:], den_sb[:])
                    for dh in range(2):
                        aon = aon_pool.tile([128, R], bf16, tag="aon")
                        nc.vector.tensor_mul(aon[:], aops[dh][:], bc[:])
                        nc.sync.dma_start(
                            ao_send[D * b + 128 * dh : D * b + 128 * dh + 128, :],
                            aon[:],
                        )
                nc.gpsimd.collective_compute(
                    "AllToAll",
                    mybir.AluOpType.bypass,
                    replica_groups=grp,
                    ins=[ao_send[:]],
                    outs=[ao_recv[:]],
                )

            big_stack.close()  # free qT/kvT/kv/mask before phase 3

            # ============ phase 3: output projection ============
            with ExitStack() as ph3:
                o_in = ph3.enter_context(tc.tile_pool(name="o_in", bufs=1))
                aoT = o_in.tile([128, 16 * R], bf16, tag="aoT")
                for k in range(16):
                    eng = nc.sync if k % 2 == 0 else nc.scalar
                    eng.dma_start(
                        aoT[:, R * k : R * k + R],
                        ao_recv[128 * k : 128 * k + 128, :],
                    )
                o_psum = ph3.enter_context(
                    tc.tile_pool(name="o_psum", bufs=4, space="PSUM")
                )
                o_out = ph3.enter_context(tc.tile_pool(name="o_out", bufs=2))
                for rc in range(4):
                    osb = o_out.tile([128, HID], f32, tag="osb")
                    for ncol in range(4):
                        ps = o_psum.tile([128, 512], f32, tag="ops")
                        for k in range(16):
                            nc.tensor.matmul(
                                ps[:],
                                aoT[:, R * k + 128 * rc : R * k + 128 * rc + 128],
                                wo_sb[
                                    :,
                                    HID * k + 512 * ncol : HID * k + 512 * ncol + 512,
                                ],
                                start=(k == 0),
                                stop=False,
                            )
                        nc.tensor.matmul(
                            ps[:],
                            ones_row[:],
                            bor_sb[:, 512 * ncol : 512 * ncol + 512],
                            start=False,
                            stop=True,
                        )
                        nc.vector.tensor_copy(
                            osb[:, 512 * ncol : 512 * ncol + 512], ps[:]
                        )
                    nc.sync.dma_start(out[128 * rc : 128 * rc + 128, :], osb[:])

    nc.compile()
    _BUILT = nc
    return nc


def _make_in_maps(x, wq, bq, wkv, bkv, wo, bo):
    import ml_dtypes

    bfnp = ml_dtypes.bfloat16
    x2d = np.asarray(x, dtype=np.float32).reshape(S, HID)
    # xT_d[c, p, hs*CH+col] = x[CH*c+col, 128*hs+p]
    xT = (
        x2d.reshape(NCH, CH, 16, 128)
        .transpose(0, 3, 2, 1)
        .reshape(NCH, 128, 16 * CH)
        .astype(bfnp)
    )
    wq3 = np.asarray(wq, dtype=np.float32).reshape(HID, NH, D)
    bq2 = np.asarray(bq, dtype=np.float32).reshape(NH, D)
    bkv1 = np.asarray(bkv, dtype=np.float32).reshape(D)
    wkv2 = np.asarray(wkv, dtype=np.float32).reshape(HID, D)
    wo2 = np.asarray(wo, dtype=np.float32).reshape(HID, HID)
    wo_h = (
        wo2.reshape(16, 128, HID).transpose(1, 0, 2).reshape(128, 16 * HID).astype(bfnp)
    )
    shared = {
        "xT": xT,
        "wo2d": wo_h,
        "bo_row": np.asarray(bo, dtype=np.float32).reshape(1, HID).astype(bfnp),
    }
    in_maps = []
    for j in range(NCORES):
        m = dict(shared)
        wq_h = wq3[:, j, :]  # [HID, D]
        qk = np.concatenate(
            [wq_h.reshape(16, 128, D), wkv2.reshape(16, 128, D)], axis=2
        )  # [16, 128, 512]
        m["wqkv"] = qk.transpose(1, 0, 2).reshape(128, 16 * 512).astype(bfnp)
        bq_h = bq2[j]
        bqkv = np.stack(
            [bq_h[:128], bq_h[128:], bkv1[:128], bkv1[128:]], axis=1
        )  # [128, 4]
        m["bqkv"] = np.ascontiguousarray(bqkv.astype(np.float32))
        in_maps.append(m)
    return in_maps


def _run(inputs, trace=False, **trace_kwargs):
    from concourse.bass_utils import run_bass_kernel_spmd

    nc = _build()
    in_maps = _make_in_maps(
        inputs["x"],
        inputs["wq"],
        inputs["bq"],
        inputs["wkv"],
        inputs["bkv"],
        inputs["wo"],
        inputs["bo"],
    )
    res = run_bass_kernel_spmd(
        nc, in_maps, list(range(NCORES)), trace=trace, **trace_kwargs
    )
    outs = [np.asarray(res.results[j]["out"]) for j in range(NCORES)]
    full = np.concatenate(outs, axis=0).reshape(1, S, HID).astype(np.float32)
    return full, res


def kernel(**inputs):
    full, _ = _run(inputs, trace=False)
    return full


if __name__ == "__main__":
    rng = np.random.default_rng(0)
    ins = {
        "x": rng.standard_normal((1, S, HID), dtype=np.float32),
        "wq": rng.standard_normal((HID, NH, D), dtype=np.float32) / 45.25,
        "bq": np.zeros((NH, D), np.float32),
        "wkv": rng.standard_normal((HID, 1, D), dtype=np.float32) / 45.25,
        "bkv": np.zeros((1, D), np.float32),
        "wo": rng.standard_normal((NH, D, HID), dtype=np.float32) / 45.25,
        "bo": np.zeros((HID,), np.float32),
        "mask": np.tril(np.ones((S, S), bool))[None, None],
    }
    out = kernel(**ins)
    print("out", out.shape, out.dtype, float(np.abs(out).max()))


# revision 33
# speedup vs baseline: 1.2235x; 1.2235x over previous
"""GQA attention kernel for Trainium2, 8 NeuronCores — v3.

Problem: B=1, S=4096, HIDDEN=2048, 8 query heads x d=256, 1 shared KV head,
causal mask, fp32 in/out.

Sharding: full tensor-parallel over heads with a replicated input.
Host-side, x is transposed/replicated to all cores in bf16 with an SBUF-
matched layout so every big load is ONE coalesced DMA (DMA dispatch on the
sync queue costs ~0.6us per instruction, so few/fat DMAs matter).
Core j owns head j end-to-end:
  1. stream xT in 4 chunks of 1024 rows; project q (own head only) and kv
     (replicated — cheaper than the AllGather it replaces) with N=1024
     bf16 matmuls; PE-transpose kv into rows-major for the AV stationary.
  2. causal flash attention for head j over all 4096 rows (fp32 PSUM, no
     max subtraction), diagonal blocks narrowed to the causal range.
  3. normalize, one bf16 AllToAll (head-major -> row-block-major).
  4. output projection of own 512-row block against full wo (N=1024).
Host concatenates the 8 row blocks.
"""

import sys

import numpy as np

sys.path.insert(0, "/opt/trn_rl_repo")

S = 4096
HID = 2048
NH = 8
D = 256
NCORES = 8
R = 512  # output rows per core / q rows per attention block
CH = 1024  # projection chunk rows
NCH = S // CH
NEG = -1.0e9
SCALE = 1.0 / 16.0  # 1/sqrt(256)

_BUILT = None


def _build():
    global _BUILT
    if _BUILT is not None:
        return _BUILT

    from contextlib import ExitStack

    import ml_dtypes

    from concourse import bacc, tile
    from concourse.bass import mybir

    dt = mybir.dt
    f32 = dt.float32
    bf16 = dt.bfloat16
    bfnp = ml_dtypes.bfloat16
    AF = mybir.ActivationFunctionType

    nc = bacc.Bacc(
        "TRN2",
        target_bir_lowering=False,
        debug=False,
        num_devices=NCORES,
    )

    # ---- DRAM I/O (host-side layouts matched to SBUF tiles) ----
    # xT_d[c, p, hs*CH+col] = x[CH*c+col, 128*hs+p]
    xT_d = nc.dram_tensor("xT", [NCH, 128, 16 * CH], bf16, kind="ExternalInput")
    # wqkv_d[p, hs*512 + j] = (wq_head | wkv)[128*hs+p, j]  (j<256 -> wq)
    wqkv_d = nc.dram_tensor("wqkv", [128, 16 * 512], bf16, kind="ExternalInput")
    # bqkv_d columns: [bq_dh0, bq_dh1, bkv_dh0, bkv_dh1]
    bqkv_d = nc.dram_tensor("bqkv", [128, 4], f32, kind="ExternalInput")
    # wo_d[p, k*HID + col] = wo2[(128*k+p), col]
    wo_d = nc.dram_tensor("wo2d", [128, 16 * HID], bf16, kind="ExternalInput")
    bo_row = nc.dram_tensor("bo_row", [1, HID], bf16, kind="ExternalInput")
    out = nc.dram_tensor("out", [R, HID], f32, kind="ExternalOutput")

    # kvx_d[p, hs*512+col] = x[512*j+col, 128*hs+p] for core j (own key slice)
    kvx_d = nc.dram_tensor("kvx", [128, 16 * 512], bf16, kind="ExternalInput")

    # ---- collective buffers ----
    grp = [list(range(NCORES))]
    ao_send = nc.dram_tensor("ao_send", [NH * D, R], bf16)
    ao_recv = nc.dram_tensor("ao_recv", [NH * D, R], bf16)
    kvT_send = nc.dram_tensor("kvT_send", [D, R], bf16)
    kvT_all = nc.dram_tensor("kvT_all", [NCORES * D, R], bf16, addr_space="Shared")

    # ---- compile-time constants ----
    fp8 = dt.float8e4
    fp8np = ml_dtypes.float8_e4m3fn
    ident_np = np.eye(128).astype(bfnp)
    ones_col_np = np.ones((128, 1)).astype(bfnp)
    ones_row_np = np.ones((1, 128)).astype(bfnp)
    ones8_np = np.ones((128, 2, 32), dtype=fp8np)
    # one shared diagonal mask: every diagonal (grel, sl) sub-block equals
    # m0[kappa, col - rel0] with m0[kappa, c] = NEG iff kappa > c
    kappa = np.arange(128)[:, None]
    cols = np.arange(512)[None, :]
    mask_np = np.where(kappa <= cols, 0.0, NEG).astype(np.float32)
    ident_d = nc.inline_tensor(ident_np, "ident")
    ones_col_d = nc.inline_tensor(ones_col_np, "ones_col")
    ones_row_d = nc.inline_tensor(ones_row_np, "ones_row")
    ones8_d = nc.inline_tensor(ones8_np, "ones8")
    mask_d = nc.inline_tensor(mask_np, "mask_const")

    with tile.TileContext(nc) as tc:
        with ExitStack() as top:
            wo_pool = top.enter_context(tc.tile_pool(name="wo", bufs=1))
            cpool = top.enter_context(tc.tile_pool(name="const", bufs=1))

            # pools alive through projections + attention (closed before
            # phase 3 to make room for aoT/osb)
            big_stack = top.enter_context(ExitStack())
            big = big_stack.enter_context(tc.tile_pool(name="big", bufs=1))
            qT = big.tile([128, 2 * S], bf16, tag="qT")  # [d-slice, rows]
            kvT = big.tile([128, 2 * S], bf16, tag="kvT")  # [d-slice, keys]
            kv_sb = big.tile([128, 32 * D], bf16, tag="kv")  # rows-major kv
            mask_sb = big.tile([128, 512], f32, tag="mask")

            with ExitStack() as ph1:
                wpool = ph1.enter_context(tc.tile_pool(name="w", bufs=1))
                xr_pool = ph1.enter_context(tc.tile_pool(name="xr", bufs=3))

                # critical-path loads first: weights + own kv x-slice on sync,
                # q x-chunks stream on the activation queue
                wqkv_sb = wpool.tile([128, 16 * 512], bf16, tag="wqkv")
                nc.sync.dma_start(wqkv_sb[:], wqkv_d[:])
                kvx_sb = wpool.tile([128, 16 * 512], bf16, tag="kvx")
                nc.sync.dma_start(kvx_sb[:], kvx_d[:])

                HCH = 8 * CH  # half-chunk: hs 0-7 / 8-15
                xrh = {}
                for c, hh in ((0, 0), (0, 1), (1, 0)):
                    t = xr_pool.tile([128, HCH], bf16, tag="xr", name=f"xr{c}_{hh}")
                    nc.scalar.dma_start(t[:], xT_d[c, :, HCH * hh : HCH * hh + HCH])
                    xrh[(c, hh)] = t

                # remaining constants
                ident = cpool.tile([128, 128], bf16, tag="ident")
                nc.sync.dma_start(ident[:], ident_d[:])
                ones_col = cpool.tile([128, 1], bf16, tag="ones_col")
                nc.sync.dma_start(ones_col[:], ones_col_d[:])
                ones_row = cpool.tile([1, 128], bf16, tag="ones_row")
                nc.sync.dma_start(ones_row[:], ones_row_d[:])
                ones8 = cpool.tile([128, 2, 32], fp8, tag="ones8")
                nc.sync.dma_start(ones8[:], ones8_d[:])
                bqkv_sb = cpool.tile([128, 4], f32, tag="bqkv")
                nc.sync.dma_start(bqkv_sb[:], bqkv_d[:])
                nc.sync.dma_start(mask_sb[:], mask_d[:])

                pj_psum = ph1.enter_context(
                    tc.tile_pool(name="pj_psum", bufs=6, space="PSUM")
                )
                tp_psum = ph1.enter_context(
                    tc.tile_pool(name="tp_psum", bufs=2, space="PSUM")
                )

                # ---- own 512-key kv projection + AllGather (overlaps q proj)
                kvs_pool = ph1.enter_context(tc.tile_pool(name="kvs", bufs=2))
                for dh in range(2):
                    kp = pj_psum.tile([128, 512], f32, tag="pj")
                    for hs in range(16):
                        nc.tensor.matmul(
                            kp[:],
                            wqkv_sb[
                                :,
                                512 * hs + 256 + 128 * dh : 512 * hs
                                + 256
                                + 128 * dh
                                + 128,
                            ],
                            kvx_sb[:, 512 * hs : 512 * hs + 512],
                            start=(hs == 0),
                            stop=(hs == 15),
                        )
                    kvs = kvs_pool.tile([128, 512], bf16, tag="kvs")
                    nc.vector.tensor_scalar_add(
                        kvs[:], kp[:], bqkv_sb[:, 2 + dh : 3 + dh]
                    )
                    nc.sync.dma_start(
                        kvT_send[128 * dh : 128 * dh + 128, :], kvs[:]
                    )
                nc.gpsimd.collective_compute(
                    "AllGather",
                    mybir.AluOpType.bypass,
                    replica_groups=grp,
                    ins=[kvT_send[:]],
                    outs=[kvT_all[:]],
                )

                # ---- q projection (own head) over streamed half-chunks
                for c in range(NCH):
                    late = [(1, 1)] if c == 1 else (
                        [(c, 0), (c, 1)] if c >= 2 else []
                    )
                    for lc, hh in late:
                        t = xr_pool.tile(
                            [128, HCH], bf16, tag="xr", name=f"xr{lc}_{hh}"
                        )
                        nc.scalar.dma_start(
                            t[:], xT_d[lc, :, HCH * hh : HCH * hh + HCH]
                        )
                        xrh[(lc, hh)] = t
                    for dh in range(2):
                        for half in range(CH // 512):
                            pp = pj_psum.tile([128, 512], f32, tag="pj")
                            for hs in range(16):
                                xt = xrh[(c, hs // 8)]
                                nc.tensor.matmul(
                                    pp[:],
                                    wqkv_sb[
                                        :,
                                        512 * hs + 128 * dh : 512 * hs + 128 * dh + 128,
                                    ],
                                    xt[
                                        :,
                                        CH * (hs % 8) + 512 * half : CH * (hs % 8)
                                        + 512 * half
                                        + 512,
                                    ],
                                    start=(hs == 0),
                                    stop=(hs == 15),
                                )
                            nc.vector.tensor_scalar_add(
                                qT[
                                    :,
                                    S * dh + CH * c + 512 * half : S * dh
                                    + CH * c
                                    + 512 * half
                                    + 512,
                                ],
                                pp[:],
                                bqkv_sb[:, dh : dh + 1],
                            )

                # ---- pull gathered kvT, build rows-major kv via PE transpose
                for src_c in range(NCORES):
                    for dh in range(2):
                        eng = nc.sync if src_c % 2 == 0 else nc.scalar
                        eng.dma_start(
                            kvT[:, S * dh + R * src_c : S * dh + R * src_c + R],
                            kvT_all[
                                D * src_c + 128 * dh : D * src_c + 128 * dh + 128, :
                            ],
                        )
                for kt in range(32):
                    for dh in range(2):
                        tp = tp_psum.tile([128, 128], bf16, tag="tp")
                        nc.tensor.transpose(
                            tp[:],
                            kvT[:, S * dh + 128 * kt : S * dh + 128 * kt + 128],
                            ident[:],
                        )
                        nc.scalar.copy(
                            kv_sb[:, D * kt + 128 * dh : D * kt + 128 * dh + 128],
                            tp[:],
                        )

            # prefetch wo during attention (one coalesced DMA)
            wo_sb = wo_pool.tile([128, 16 * HID], bf16, tag="wo")
            nc.scalar.dma_start(wo_sb[:], wo_d[:])

            # ============ phase 2: causal flash attention ============
            with ExitStack() as ph2:
                s_psum = ph2.enter_context(
                    tc.tile_pool(name="s_psum", bufs=3, space="PSUM")
                )
                ao_psum = ph2.enter_context(
                    tc.tile_pool(name="ao_psum", bufs=3, space="PSUM")
                )
                den_psum = ph2.enter_context(
                    tc.tile_pool(name="den_psum", bufs=2, space="PSUM")
                )
                p_pool = ph2.enter_context(tc.tile_pool(name="p", bufs=3))
                p8_pool = ph2.enter_context(tc.tile_pool(name="p8", bufs=3))
                nrm_pool = ph2.enter_context(tc.tile_pool(name="nrm", bufs=2))
                aon_pool = ph2.enter_context(tc.tile_pool(name="aon", bufs=4))

                for b in range(8):
                    aops = [
                        ao_psum.tile([128, R], f32, tag="aops", name=f"aops{b}_{i}")
                        for i in range(2)
                    ]
                    denp = den_psum.tile([32, R], f32, tag="denp")
                    ngroups = 2 * (b + 1)

                    def consume(item):
                        kg, sl, r0, pt_c, pt8_c = item
                        k = 2 * kg + sl
                        diag_c = kg >= 2 * b
                        first = kg == 0 and sl == 0
                        last = kg == ngroups - 1 and sl == 1
                        if diag_c:
                            nc.tensor.matmul(
                                denp[0:1, r0:R],
                                ones_col[:],
                                pt_c[:, 512 * sl + r0 : 512 * sl + 512],
                                start=first,
                                stop=last,
                                skip_group_check=True,
                            )
                        for dh in range(2):
                            nc.tensor.matmul(
                                aops[dh][:, r0:R],
                                kv_sb[:, D * k + 128 * dh : D * k + 128 * dh + 128],
                                pt_c[:, 512 * sl + r0 : 512 * sl + 512],
                                start=first,
                                stop=last,
                            )
                        if (not diag_c) and sl == 1:
                            # denominator: one fp8 DoubleRow matmul per group
                            # (den is a coherent positive sum — fp8 rounding
                            # averages out)
                            nc.tensor.matmul(
                                denp[:, 0:R],
                                ones8[:],
                                pt8_c[:],
                                start=(kg == 0),
                                stop=False,
                                perf_mode=mybir.MatmulPerfMode.DoubleRow,
                                skip_group_check=True,
                            )

                    prev = None
                    pt = None
                    pt8 = None
                    for kg in range(ngroups):
                        diag = kg >= 2 * b
                        grel = kg - 2 * b
                        for sl in range(2):
                            r0 = 256 * grel + 128 * sl if diag else 0
                            k = 2 * kg + sl
                            st = s_psum.tile([128, 512], f32, tag="st")
                            for dh in range(2):
                                nc.tensor.matmul(
                                    st[:, r0:512],
                                    kvT[:, S * dh + 128 * k : S * dh + 128 * k + 128],
                                    qT[:, S * dh + R * b + r0 : S * dh + R * b + R],
                                    start=(dh == 0),
                                    stop=(dh == 1),
                                )
                            if diag:
                                nc.vector.tensor_add(
                                    st[:, r0:512],
                                    st[:, r0:512],
                                    mask_sb[:, 0 : 512 - r0],
                                )
                            if sl == 0:
                                pt = p_pool.tile([128, 1024], bf16, tag="pt")
                                pt8 = (
                                    None
                                    if diag
                                    else p8_pool.tile([128, 2, 512], fp8, tag="pt8")
                                )
                            nc.scalar.activation(
                                pt[:, 512 * sl + r0 : 512 * sl + 512],
                                st[:, r0:512],
                                AF.Exp,
                                scale=SCALE,
                            )
                            if not diag:
                                nc.scalar.activation(
                                    pt8[:, sl, :],
                                    st[:, 0:512],
                                    AF.Exp,
                                    scale=SCALE,
                                )
                            if prev is not None:
                                consume(prev)
                            prev = (kg, sl, r0, pt, pt8)
                    consume(prev)
                    # normalize + send (bf16)
                    den_sb = nrm_pool.tile([1, R], f32, tag="den_sb")
                    nc.vector.reciprocal(den_sb[:], denp[0:1, :])
                    bc = nrm_pool.tile([128, R], f32, tag="bc")
                    nc.gpsimd.partition_broadcast(bc[:], den_sb[:])
                    for dh in range(2):
                        aon = aon_pool.tile([128, R], bf16, tag="aon")
                        nc.vector.tensor_mul(aon[:], aops[dh][:], bc[:])
                        nc.sync.dma_start(
                            ao_send[D * b + 128 * dh : D * b + 128 * dh + 128, :],
                            aon[:],
                        )
                nc.gpsimd.collective_compute(
                    "AllToAll",
                    mybir.AluOpType.bypass,
                    replica_groups=grp,
                    ins=[ao_send[:]],
                    outs=[ao_recv[:]],
                )

            big_stack.close()  # free qT/kvT/kv/mask before phase 3

            # ============ phase 3: output projection ============
            with ExitStack() as ph3:
                o_in = ph3.enter_context(tc.tile_pool(name="o_in", bufs=1))
                bor_sb = o_in.tile([1, HID], bf16, tag="bor")
                nc.sync.dma_start(bor_sb[:], bo_row[:])
                # 4 aoT tiles so the first out-proj matmuls start after the
                # first quarter of the loads lands
                aoTs = []
                for g in range(4):
                    t = o_in.tile([128, 4 * R], bf16, tag=f"aoT{g}")
                    for kk in range(4):
                        k = 4 * g + kk
                        eng = nc.sync if k % 2 == 0 else nc.scalar
                        eng.dma_start(
                            t[:, R * kk : R * kk + R],
                            ao_recv[128 * k : 128 * k + 128, :],
                        )
                    aoTs.append(t)
                o_psum = ph3.enter_context(
                    tc.tile_pool(name="o_psum", bufs=4, space="PSUM")
                )
                o_out = ph3.enter_context(tc.tile_pool(name="o_out", bufs=2))
                for rc in range(4):
                    osb = o_out.tile([128, HID], f32, tag="osb")
                    for ncol in range(4):
                        ps = o_psum.tile([128, 512], f32, tag="ops")
                        for k in range(16):
                            nc.tensor.matmul(
                                ps[:],
                                aoTs[k // 4][:, R * (k % 4) + 128 * rc : R * (k % 4) + 128 * rc + 128],
                                wo_sb[
                                    :,
                                    HID * k + 512 * ncol : HID * k + 512 * ncol + 512,
                                ],
                                start=(k == 0),
                                stop=False,
                            )
                        nc.tensor.matmul(
                            ps[:],
                            ones_row[:],
                            bor_sb[:, 512 * ncol : 512 * ncol + 512],
                            start=False,
                            stop=True,
                        )
                        nc.vector.tensor_copy(
                            osb[:, 512 * ncol : 512 * ncol + 512], ps[:]
                        )
                    nc.sync.dma_start(out[128 * rc : 128 * rc + 128, :], osb[:])

    nc.compile()
    _BUILT = nc
    return nc


def _make_in_maps(x, wq, bq, wkv, bkv, wo, bo):
    import ml_dtypes

    bfnp = ml_dtypes.bfloat16
    x2d = np.asarray(x, dtype=np.float32).reshape(S, HID)
    # xT_d[c, p, hs*CH+col] = x[CH*c+col, 128*hs+p]
    xT = (
        x2d.reshape(NCH, CH, 16, 128)
        .transpose(0, 3, 2, 1)
        .reshape(NCH, 128, 16 * CH)
        .astype(bfnp)
    )
    wq3 = np.asarray(wq, dtype=np.float32).reshape(HID, NH, D)
    bq2 = np.asarray(bq, dtype=np.float32).reshape(NH, D)
    bkv1 = np.asarray(bkv, dtype=np.float32).reshape(D)
    wkv2 = np.asarray(wkv, dtype=np.float32).reshape(HID, D)
    wo2 = np.asarray(wo, dtype=np.float32).reshape(HID, HID)
    wo_h = (
        wo2.reshape(16, 128, HID).transpose(1, 0, 2).reshape(128, 16 * HID).astype(bfnp)
    )
    shared = {
        "xT": xT,
        "wo2d": wo_h,
        "bo_row": np.asarray(bo, dtype=np.float32).reshape(1, HID).astype(bfnp),
    }
    # kvx[p, hs*512+col] = x[512*j+col, 128*hs+p]
    xr4 = x2d.reshape(NCORES, 512, 16, 128)  # [j, col, hs, p]
    in_maps = []
    for j in range(NCORES):
        m = dict(shared)
        m["kvx"] = (
            np.ascontiguousarray(xr4[j].transpose(2, 1, 0))
            .reshape(128, 16 * 512)
            .astype(bfnp)
        )
        wq_h = wq3[:, j, :]  # [HID, D]
        qk = np.concatenate(
            [wq_h.reshape(16, 128, D), wkv2.reshape(16, 128, D)], axis=2
        )  # [16, 128, 512]
        m["wqkv"] = qk.transpose(1, 0, 2).reshape(128, 16 * 512).astype(bfnp)
        bq_h = bq2[j]
        bqkv = np.stack(
            [bq_h[:128], bq_h[128:], bkv1[:128], bkv1[128:]], axis=1
        )  # [128, 4]
        m["bqkv"] = np.ascontiguousarray(bqkv.astype(np.float32))
        in_maps.append(m)
    return in_maps


def _run(inputs, trace=False, **trace_kwargs):
    from concourse.bass_utils import run_bass_kernel_spmd

    nc = _build()
    in_maps = _make_in_maps(
        inputs["x"],
        inputs["wq"],
        inputs["bq"],
        inputs["wkv"],
        inputs["bkv"],
        inputs["wo"],
        inputs["bo"],
    )
    res = run_bass_kernel_spmd(
        nc, in_maps, list(range(NCORES)), trace=trace, **trace_kwargs
    )
    outs = [np.asarray(res.results[j]["out"]) for j in range(NCORES)]
    full = np.concatenate(outs, axis=0).reshape(1, S, HID).astype(np.float32)
    return full, res


def kernel(**inputs):
    full, _ = _run(inputs, trace=False)
    return full


if __name__ == "__main__":
    rng = np.random.default_rng(0)
    ins = {
        "x": rng.standard_normal((1, S, HID), dtype=np.float32),
        "wq": rng.standard_normal((HID, NH, D), dtype=np.float32) / 45.25,
        "bq": np.zeros((NH, D), np.float32),
        "wkv": rng.standard_normal((HID, 1, D), dtype=np.float32) / 45.25,
        "bkv": np.zeros((1, D), np.float32),
        "wo": rng.standard_normal((NH, D, HID), dtype=np.float32) / 45.25,
        "bo": np.zeros((HID,), np.float32),
        "mask": np.tril(np.ones((S, S), bool))[None, None],
    }
    out = kernel(**ins)
    print("out", out.shape, out.dtype, float(np.abs(out).max()))


# revision 35
# speedup vs baseline: 1.3242x; 1.0823x over previous
"""GQA attention kernel for Trainium2, 8 NeuronCores — v3.

Problem: B=1, S=4096, HIDDEN=2048, 8 query heads x d=256, 1 shared KV head,
causal mask, fp32 in/out.

Sharding: full tensor-parallel over heads with a replicated input.
Host-side, x is transposed/replicated to all cores in bf16 with an SBUF-
matched layout so every big load is ONE coalesced DMA (DMA dispatch on the
sync queue costs ~0.6us per instruction, so few/fat DMAs matter).
Core j owns head j end-to-end:
  1. stream xT in 4 chunks of 1024 rows; project q (own head only) and kv
     (replicated — cheaper than the AllGather it replaces) with N=1024
     bf16 matmuls; PE-transpose kv into rows-major for the AV stationary.
  2. causal flash attention for head j over all 4096 rows (fp32 PSUM, no
     max subtraction), diagonal blocks narrowed to the causal range.
  3. normalize, one bf16 AllToAll (head-major -> row-block-major).
  4. output projection of own 512-row block against full wo (N=1024).
Host concatenates the 8 row blocks.
"""

import sys

import numpy as np

sys.path.insert(0, "/opt/trn_rl_repo")

S = 4096
HID = 2048
NH = 8
D = 256
NCORES = 8
R = 512  # output rows per core / q rows per attention block
CH = 1024  # projection chunk rows
NCH = S // CH
NEG = -1.0e9
SCALE = 1.0 / 16.0  # 1/sqrt(256)

_BUILT = None


def _build():
    global _BUILT
    if _BUILT is not None:
        return _BUILT

    from contextlib import ExitStack

    import ml_dtypes

    from concourse import bacc, tile
    from concourse.bass import mybir

    dt = mybir.dt
    f32 = dt.float32
    bf16 = dt.bfloat16
    bfnp = ml_dtypes.bfloat16
    AF = mybir.ActivationFunctionType

    nc = bacc.Bacc(
        "TRN2",
        target_bir_lowering=False,
        debug=False,
        num_devices=NCORES,
    )

    # ---- DRAM I/O (host-side layouts matched to SBUF tiles) ----
    # xT_d[c, p, hs*CH+col] = x[CH*c+col, 128*hs+p]
    xT_d = nc.dram_tensor("xT", [NCH, 128, 16 * CH], bf16, kind="ExternalInput")
    # wqkv_d[p, hs*512 + j] = (wq_head | wkv)[128*hs+p, j]  (j<256 -> wq)
    wqkv_d = nc.dram_tensor("wqkv", [128, 16 * 512], bf16, kind="ExternalInput")
    # bqkv_d columns: [bq_dh0, bq_dh1, bkv_dh0, bkv_dh1]
    bqkv_d = nc.dram_tensor("bqkv", [128, 4], f32, kind="ExternalInput")
    # wo_d[p, k*HID + col] = wo2[(128*k+p), col]
    wo_d = nc.dram_tensor("wo2d", [128, 16 * HID], bf16, kind="ExternalInput")
    bo_row = nc.dram_tensor("bo_row", [1, HID], bf16, kind="ExternalInput")
    out = nc.dram_tensor("out", [R, HID], f32, kind="ExternalOutput")

    # kvx_d[p, hs*512+col] = x[512*j+col, 128*hs+p] for core j (own key slice)
    kvx_d = nc.dram_tensor("kvx", [128, 16 * 512], bf16, kind="ExternalInput")

    # ---- collective buffers ----
    grp = [list(range(NCORES))]
    ao_send = nc.dram_tensor("ao_send", [NH * D, R], bf16)
    ao_recv = nc.dram_tensor("ao_recv", [NH * D, R], bf16)
    kvT_send = nc.dram_tensor("kvT_send", [D, R], bf16)
    kvT_all = nc.dram_tensor("kvT_all", [NCORES * D, R], bf16, addr_space="Shared")

    # ---- compile-time constants ----
    fp8 = dt.float8e4
    fp8np = ml_dtypes.float8_e4m3fn
    ident_np = np.eye(128).astype(bfnp)
    ones_col_np = np.ones((128, 1)).astype(bfnp)
    ones_row_np = np.ones((1, 128)).astype(bfnp)
    ones8_np = np.ones((128, 2, 32), dtype=fp8np)
    # one shared diagonal mask: every diagonal (grel, sl) sub-block equals
    # m0[kappa, col - rel0] with m0[kappa, c] = NEG iff kappa > c
    kappa = np.arange(128)[:, None]
    cols = np.arange(512)[None, :]
    mask_np = np.where(kappa <= cols, 0.0, NEG).astype(np.float32)
    ident_d = nc.inline_tensor(ident_np, "ident")
    ones_col_d = nc.inline_tensor(ones_col_np, "ones_col")
    ones_row_d = nc.inline_tensor(ones_row_np, "ones_row")
    ones8_d = nc.inline_tensor(ones8_np, "ones8")
    mask_d = nc.inline_tensor(mask_np, "mask_const")

    with tile.TileContext(nc) as tc:
        with ExitStack() as top:
            wo_pool = top.enter_context(tc.tile_pool(name="wo", bufs=1))
            cpool = top.enter_context(tc.tile_pool(name="const", bufs=1))

            # pools alive through projections + attention (closed before
            # phase 3 to make room for aoT/osb)
            big_stack = top.enter_context(ExitStack())
            big = big_stack.enter_context(tc.tile_pool(name="big", bufs=1))
            qT = big.tile([128, 2 * S], bf16, tag="qT")  # [d-slice, rows]
            kvT = big.tile([128, 2 * S], bf16, tag="kvT")  # [d-slice, keys]
            kv_sb = big.tile([128, 32 * D], bf16, tag="kv")  # rows-major kv
            mask_sb = big.tile([128, 512], f32, tag="mask")

            with ExitStack() as ph1:
                wpool = ph1.enter_context(tc.tile_pool(name="w", bufs=1))
                xr_pool = ph1.enter_context(tc.tile_pool(name="xr", bufs=3))

                # critical-path loads first: weights + own kv x-slice on sync,
                # q x-chunks stream on the activation queue
                wqkv_sb = wpool.tile([128, 16 * 512], bf16, tag="wqkv")
                nc.sync.dma_start(wqkv_sb[:], wqkv_d[:])
                kvx_sb = wpool.tile([128, 16 * 512], bf16, tag="kvx")
                nc.sync.dma_start(kvx_sb[:], kvx_d[:])

                HCH = 8 * CH  # half-chunk: hs 0-7 / 8-15
                xrh = {}
                for c, hh in ((0, 0), (0, 1), (1, 0)):
                    t = xr_pool.tile([128, HCH], bf16, tag="xr", name=f"xr{c}_{hh}")
                    eng = nc.scalar if hh == 0 else nc.sync
                    eng.dma_start(t[:], xT_d[c, :, HCH * hh : HCH * hh + HCH])
                    xrh[(c, hh)] = t

                # remaining constants
                ident = cpool.tile([128, 128], bf16, tag="ident")
                nc.sync.dma_start(ident[:], ident_d[:])
                ones_col = cpool.tile([128, 1], bf16, tag="ones_col")
                nc.sync.dma_start(ones_col[:], ones_col_d[:])
                ones_row = cpool.tile([1, 128], bf16, tag="ones_row")
                nc.sync.dma_start(ones_row[:], ones_row_d[:])
                ones8 = cpool.tile([128, 2, 32], fp8, tag="ones8")
                nc.sync.dma_start(ones8[:], ones8_d[:])
                bqkv_sb = cpool.tile([128, 4], f32, tag="bqkv")
                nc.sync.dma_start(bqkv_sb[:], bqkv_d[:])
                nc.sync.dma_start(mask_sb[:], mask_d[:])

                pj_psum = ph1.enter_context(
                    tc.tile_pool(name="pj_psum", bufs=6, space="PSUM")
                )
                tp_psum = ph1.enter_context(
                    tc.tile_pool(name="tp_psum", bufs=2, space="PSUM")
                )

                # ---- own 512-key kv projection + AllGather (overlaps q proj)
                kvs_pool = ph1.enter_context(tc.tile_pool(name="kvs", bufs=2))
                for dh in range(2):
                    kp = pj_psum.tile([128, 512], f32, tag="pj")
                    for hs in range(16):
                        nc.tensor.matmul(
                            kp[:],
                            wqkv_sb[
                                :,
                                512 * hs + 256 + 128 * dh : 512 * hs
                                + 256
                                + 128 * dh
                                + 128,
                            ],
                            kvx_sb[:, 512 * hs : 512 * hs + 512],
                            start=(hs == 0),
                            stop=(hs == 15),
                        )
                    kvs = kvs_pool.tile([128, 512], bf16, tag="kvs")
                    nc.vector.tensor_scalar_add(
                        kvs[:], kp[:], bqkv_sb[:, 2 + dh : 3 + dh]
                    )
                    nc.sync.dma_start(
                        kvT_send[128 * dh : 128 * dh + 128, :], kvs[:]
                    )
                nc.gpsimd.collective_compute(
                    "AllGather",
                    mybir.AluOpType.bypass,
                    replica_groups=grp,
                    ins=[kvT_send[:]],
                    outs=[kvT_all[:]],
                )

                # ---- q projection (own head) over streamed half-chunks
                for c in range(NCH):
                    late = [(1, 1)] if c == 1 else (
                        [(c, 0), (c, 1)] if c >= 2 else []
                    )
                    for lc, hh in late:
                        t = xr_pool.tile(
                            [128, HCH], bf16, tag="xr", name=f"xr{lc}_{hh}"
                        )
                        eng = nc.scalar if hh == 0 else nc.sync
                        eng.dma_start(
                            t[:], xT_d[lc, :, HCH * hh : HCH * hh + HCH]
                        )
                        xrh[(lc, hh)] = t
                    for dh in range(2):
                        for half in range(CH // 512):
                            pp = pj_psum.tile([128, 512], f32, tag="pj")
                            for hs in range(16):
                                xt = xrh[(c, hs // 8)]
                                nc.tensor.matmul(
                                    pp[:],
                                    wqkv_sb[
                                        :,
                                        512 * hs + 128 * dh : 512 * hs + 128 * dh + 128,
                                    ],
                                    xt[
                                        :,
                                        CH * (hs % 8) + 512 * half : CH * (hs % 8)
                                        + 512 * half
                                        + 512,
                                    ],
                                    start=(hs == 0),
                                    stop=(hs == 15),
                                )
                            nc.vector.tensor_scalar_add(
                                qT[
                                    :,
                                    S * dh + CH * c + 512 * half : S * dh
                                    + CH * c
                                    + 512 * half
                                    + 512,
                                ],
                                pp[:],
                                bqkv_sb[:, dh : dh + 1],
                            )

                # ---- pull gathered kvT, build rows-major kv via PE transpose
                for src_c in range(NCORES):
                    for dh in range(2):
                        eng = nc.sync if src_c % 2 == 0 else nc.scalar
                        eng.dma_start(
                            kvT[:, S * dh + R * src_c : S * dh + R * src_c + R],
                            kvT_all[
                                D * src_c + 128 * dh : D * src_c + 128 * dh + 128, :
                            ],
                        )
                for kt in range(32):
                    for dh in range(2):
                        tp = tp_psum.tile([128, 128], bf16, tag="tp")
                        nc.tensor.transpose(
                            tp[:],
                            kvT[:, S * dh + 128 * kt : S * dh + 128 * kt + 128],
                            ident[:],
                        )
                        nc.scalar.copy(
                            kv_sb[:, D * kt + 128 * dh : D * kt + 128 * dh + 128],
                            tp[:],
                        )

            # prefetch wo during attention (one coalesced DMA)
            wo_sb = wo_pool.tile([128, 16 * HID], bf16, tag="wo")
            nc.scalar.dma_start(wo_sb[:], wo_d[:])

            # ============ phase 2: causal flash attention ============
            with ExitStack() as ph2:
                s_psum = ph2.enter_context(
                    tc.tile_pool(name="s_psum", bufs=3, space="PSUM")
                )
                ao_psum = ph2.enter_context(
                    tc.tile_pool(name="ao_psum", bufs=4, space="PSUM")
                )
                den_psum = ph2.enter_context(
                    tc.tile_pool(name="den_psum", bufs=1, space="PSUM")
                )
                p_pool = ph2.enter_context(tc.tile_pool(name="p", bufs=3))
                p8_pool = ph2.enter_context(tc.tile_pool(name="p8", bufs=3))
                nrm_pool = ph2.enter_context(tc.tile_pool(name="nrm", bufs=2))
                aon_pool = ph2.enter_context(tc.tile_pool(name="aon", bufs=4))

                for b in range(8):
                    aops = [
                        ao_psum.tile([128, R], f32, tag="aops", name=f"aops{b}_{i}")
                        for i in range(2)
                    ]
                    denp = den_psum.tile([32, R], f32, tag="denp")
                    ngroups = 2 * (b + 1)

                    def consume(item):
                        kg, sl, r0, pt_c, pt8_c = item
                        k = 2 * kg + sl
                        diag_c = kg >= 2 * b
                        first = kg == 0 and sl == 0
                        last = kg == ngroups - 1 and sl == 1
                        if diag_c:
                            nc.tensor.matmul(
                                denp[0:1, r0:R],
                                ones_col[:],
                                pt_c[:, 512 * sl + r0 : 512 * sl + 512],
                                start=first,
                                stop=last,
                                skip_group_check=True,
                            )
                        for dh in range(2):
                            nc.tensor.matmul(
                                aops[dh][:, r0:R],
                                kv_sb[:, D * k + 128 * dh : D * k + 128 * dh + 128],
                                pt_c[:, 512 * sl + r0 : 512 * sl + 512],
                                start=first,
                                stop=last,
                            )
                        if (not diag_c) and sl == 1:
                            # denominator: one fp8 DoubleRow matmul per group
                            # (den is a coherent positive sum — fp8 rounding
                            # averages out)
                            nc.tensor.matmul(
                                denp[:, 0:R],
                                ones8[:],
                                pt8_c[:],
                                start=(kg == 0),
                                stop=False,
                                perf_mode=mybir.MatmulPerfMode.DoubleRow,
                                skip_group_check=True,
                            )

                    prev = None
                    pt = None
                    pt8 = None
                    for kg in range(ngroups):
                        diag = kg >= 2 * b
                        grel = kg - 2 * b
                        for sl in range(2):
                            r0 = 256 * grel + 128 * sl if diag else 0
                            k = 2 * kg + sl
                            st = s_psum.tile([128, 512], f32, tag="st")
                            for dh in range(2):
                                nc.tensor.matmul(
                                    st[:, r0:512],
                                    kvT[:, S * dh + 128 * k : S * dh + 128 * k + 128],
                                    qT[:, S * dh + R * b + r0 : S * dh + R * b + R],
                                    start=(dh == 0),
                                    stop=(dh == 1),
                                )
                            if diag:
                                nc.vector.tensor_add(
                                    st[:, r0:512],
                                    st[:, r0:512],
                                    mask_sb[:, 0 : 512 - r0],
                                )
                            if sl == 0:
                                pt = p_pool.tile([128, 1024], bf16, tag="pt")
                                pt8 = (
                                    None
                                    if diag
                                    else p8_pool.tile([128, 2, 512], fp8, tag="pt8")
                                )
                            nc.scalar.activation(
                                pt[:, 512 * sl + r0 : 512 * sl + 512],
                                st[:, r0:512],
                                AF.Exp,
                                scale=SCALE,
                            )
                            if not diag:
                                nc.scalar.activation(
                                    pt8[:, sl, :],
                                    st[:, 0:512],
                                    AF.Exp,
                                    scale=SCALE,
                                )
                            if prev is not None:
                                consume(prev)
                            prev = (kg, sl, r0, pt, pt8)
                    consume(prev)
                    # normalize + send (bf16)
                    den_sb = nrm_pool.tile([1, R], f32, tag="den_sb")
                    nc.vector.reciprocal(den_sb[:], denp[0:1, :])
                    bc = nrm_pool.tile([128, R], f32, tag="bc")
                    nc.gpsimd.partition_broadcast(bc[:], den_sb[:])
                    for dh in range(2):
                        aon = aon_pool.tile([128, R], bf16, tag="aon")
                        nc.vector.tensor_mul(aon[:], aops[dh][:], bc[:])
                        nc.sync.dma_start(
                            ao_send[D * b + 128 * dh : D * b + 128 * dh + 128, :],
                            aon[:],
                        )
                nc.gpsimd.collective_compute(
                    "AllToAll",
                    mybir.AluOpType.bypass,
                    replica_groups=grp,
                    ins=[ao_send[:]],
                    outs=[ao_recv[:]],
                )

            big_stack.close()  # free qT/kvT/kv/mask before phase 3

            # ============ phase 3: output projection ============
            with ExitStack() as ph3:
                o_in = ph3.enter_context(tc.tile_pool(name="o_in", bufs=1))
                bor_sb = o_in.tile([1, HID], bf16, tag="bor")
                nc.sync.dma_start(bor_sb[:], bo_row[:])
                bo_bc = o_in.tile([128, HID], bf16, tag="bo_bc")
                nc.gpsimd.partition_broadcast(bo_bc[:], bor_sb[:])
                # 4 aoT tiles so the first out-proj matmuls start after the
                # first quarter of the loads lands
                aoTs = []
                for g in range(4):
                    t = o_in.tile([128, 4 * R], bf16, tag=f"aoT{g}")
                    for kk in range(4):
                        k = 4 * g + kk
                        eng = nc.sync if k % 2 == 0 else nc.scalar
                        eng.dma_start(
                            t[:, R * kk : R * kk + R],
                            ao_recv[128 * k : 128 * k + 128, :],
                        )
                    aoTs.append(t)
                o_psum = ph3.enter_context(
                    tc.tile_pool(name="o_psum", bufs=4, space="PSUM")
                )
                o_out = ph3.enter_context(tc.tile_pool(name="o_out", bufs=2))
                for rc in range(4):
                    osb = o_out.tile([128, HID], f32, tag="osb")
                    for ncol in range(4):
                        ps = o_psum.tile([128, 512], f32, tag="ops")
                        for k in range(16):
                            nc.tensor.matmul(
                                ps[:],
                                aoTs[k // 4][:, R * (k % 4) + 128 * rc : R * (k % 4) + 128 * rc + 128],
                                wo_sb[
                                    :,
                                    HID * k + 512 * ncol : HID * k + 512 * ncol + 512,
                                ],
                                start=(k == 0),
                                stop=(k == 15),
                            )
                        nc.vector.tensor_add(
                            osb[:, 512 * ncol : 512 * ncol + 512],
                            ps[:],
                            bo_bc[:, 512 * ncol : 512 * ncol + 512],
                        )
                    nc.sync.dma_start(out[128 * rc : 128 * rc + 128, :], osb[:])

    nc.compile()
    _BUILT = nc
    return nc


def _make_in_maps(x, wq, bq, wkv, bkv, wo, bo):
    import ml_dtypes

    bfnp = ml_dtypes.bfloat16
    x2d = np.asarray(x, dtype=np.float32).reshape(S, HID)
    # xT_d[c, p, hs*CH+col] = x[CH*c+col, 128*hs+p]
    xT = (
        x2d.reshape(NCH, CH, 16, 128)
        .transpose(0, 3, 2, 1)
        .reshape(NCH, 128, 16 * CH)
        .astype(bfnp)
    )
    wq3 = np.asarray(wq, dtype=np.float32).reshape(HID, NH, D)
    bq2 = np.asarray(bq, dtype=np.float32).reshape(NH, D)
    bkv1 = np.asarray(bkv, dtype=np.float32).reshape(D)
    wkv2 = np.asarray(wkv, dtype=np.float32).reshape(HID, D)
    wo2 = np.asarray(wo, dtype=np.float32).reshape(HID, HID)
    wo_h = (
        wo2.reshape(16, 128, HID).transpose(1, 0, 2).reshape(128, 16 * HID).astype(bfnp)
    )
    shared = {
        "xT": xT,
        "wo2d": wo_h,
        "bo_row": np.asarray(bo, dtype=np.float32).reshape(1, HID).astype(bfnp),
    }
    # kvx[p, hs*512+col] = x[512*j+col, 128*hs+p]
    xr4 = x2d.reshape(NCORES, 512, 16, 128)  # [j, col, hs, p]
    in_maps = []
    for j in range(NCORES):
        m = dict(shared)
        m["kvx"] = (
            np.ascontiguousarray(xr4[j].transpose(2, 1, 0))
            .reshape(128, 16 * 512)
            .astype(bfnp)
        )
        wq_h = wq3[:, j, :]  # [HID, D]
        qk = np.concatenate(
            [wq_h.reshape(16, 128, D), wkv2.reshape(16, 128, D)], axis=2
        )  # [16, 128, 512]
        m["wqkv"] = qk.transpose(1, 0, 2).reshape(128, 16 * 512).astype(bfnp)
        bq_h = bq2[j]
        bqkv = np.stack(
            [bq_h[:128], bq_h[128:], bkv1[:128], bkv1[128:]], axis=1
        )  # [128, 4]
        m["bqkv"] = np.ascontiguousarray(bqkv.astype(np.float32))
        in_maps.append(m)
    return in_maps


def _run(inputs, trace=False, **trace_kwargs):
    from concourse.bass_utils import run_bass_kernel_spmd

    nc = _build()
    in_maps = _make_in_maps(
        inputs["x"],
        inputs["wq"],
        inputs["bq"],
        inputs["wkv"],
        inputs["bkv"],
        inputs["wo"],
        inputs["bo"],
    )
    res = run_bass_kernel_spmd(
        nc, in_maps, list(range(NCORES)), trace=trace, **trace_kwargs
    )
    outs = [np.asarray(res.results[j]["out"]) for j in range(NCORES)]
    full = np.concatenate(outs, axis=0).reshape(1, S, HID).astype(np.float32)
    return full, res


def kernel(**inputs):
    full, _ = _run(inputs, trace=False)
    return full


if __name__ == "__main__":
    rng = np.random.default_rng(0)
    ins = {
        "x": rng.standard_normal((1, S, HID), dtype=np.float32),
        "wq": rng.standard_normal((HID, NH, D), dtype=np.float32) / 45.25,
        "bq": np.zeros((NH, D), np.float32),
        "wkv": rng.standard_normal((HID, 1, D), dtype=np.float32) / 45.25,
        "bkv": np.zeros((1, D), np.float32),
        "wo": rng.standard_normal((NH, D, HID), dtype=np.float32) / 45.25,
        "bo": np.zeros((HID,), np.float32),
        "mask": np.tril(np.ones((S, S), bool))[None, None],
    }
    out = kernel(**ins)
    print("out", out.shape, out.dtype, float(np.abs(out).max()))
